# revision 1
# baseline (speedup 1.0000x reference)
"""Trainium2 Bass kernel for the LstmPredictor module.

Model (per batch element b):
    h   = relu(x @ w_in_k + w_in_b)            # (T=20, 64)
    enc = LSTM_256(h)[-1]                      # (256,)
    dec = LSTM_256(repeat(enc, 15))            # (15, 256)  (return_seq)
    out = [dec @ mean_k + mean_b, relu(dec @ lv_k + lv_b)]   # (15, 4)

Strategy: pure data parallel over batch (8192 -> 8 cores x 1024).
All on-chip tensors keep batch in the SBUF free dimension ("transposed"
layout) so the recurrent state hT (2 x 128 partitions, 1024 free) is
directly the moving operand of the next step's matmuls. Matmuls run in
float32r (full fp32 storage, reduced-precision multiply, 1 col/cycle).

Per LSTM step (B=1024 per core, split into 2 free-dim chunks of 512):
  PSUM banks [i i f f | g g | o o] accumulate
     z = enc_k_ext.T @ xh_t  (K=65, bias folded via ones row)
       + enc_rk[0:128].T @ hT[0] + enc_rk[128:256].T @ hT[1]
  ACT drains them with batched Sigmoid/Tanh calls, DVE updates c and h.
The decoder input projection (repeated enc) is precomputed once (zdx)
and injected per step with an identity matmul; the output head folds its
bias via a K=1 ones-row matmul, mean goes straight to DRAM, log_var is
relu'd in one batched end phase.
"""

import numpy as np

import concourse.bass as bass
import concourse.mybir as mybir
import concourse.tile as tile
from concourse import bacc, bass_utils
from concourse.bass import ds, ts

N_CORES = 8
B_FULL = 8192
BC = B_FULL // N_CORES  # 1024 batch per core
NCH = 2  # free-dim chunks of 512
CW = BC // NCH  # 512
T_ENC = 20
T_DEC = 15
H = 256
FH = 64  # input-projection width
DT = mybir.dt.float32r
F32 = mybir.dt.float32
AF = mybir.ActivationFunctionType

LAST_RESULTS = None  # BassKernelResults of the most recent run (for test.py)
_NC_CACHE = []


def _build_nc():
    nc = bacc.Bacc("TRN2", target_bir_lowering=False, debug=False, num_devices=N_CORES)

    # ---- DRAM I/O (per-core shapes; host marshals layouts) ----
    xt_d = nc.dram_tensor("xt", [T_ENC, 8, BC], DT, kind="ExternalInput")
    wink_d = nc.dram_tensor("w_in_k", [8, FH], DT, kind="ExternalInput")
    winb_d = nc.dram_tensor("w_in_b128", [128, 1], F32, kind="ExternalInput")
    enck_d = nc.dram_tensor("enc_k_ext", [65, 4 * H], DT, kind="ExternalInput")
    encrk_d = nc.dram_tensor("enc_rk", [2, 128, 4 * H], DT, kind="ExternalInput")
    deck_d = nc.dram_tensor("dec_k", [2, 128, 4 * H], DT, kind="ExternalInput")
    deckb_d = nc.dram_tensor("dec_b", [1, 4 * H], DT, kind="ExternalInput")
    decrk_d = nc.dram_tensor("dec_rk", [2, 128, 4 * H], DT, kind="ExternalInput")
    whead_d = nc.dram_tensor("w_head", [2, 128, 4], DT, kind="ExternalInput")
    hbias_d = nc.dram_tensor("head_bias", [1, 4], DT, kind="ExternalInput")
    ident_d = nc.dram_tensor("ident", [128, 128], DT, kind="ExternalInput")
    ones_d = nc.dram_tensor("ones", [1, BC], DT, kind="ExternalInput")
    zeros_d = nc.dram_tensor("zeros", [128, 2, BC], DT, kind="ExternalInput")

    om_d = nc.dram_tensor("out_mean", [T_DEC * 4, CW], F32, kind="ExternalOutput")
    ol_d = nc.dram_tensor("out_lv", [T_DEC * 4, CW], F32, kind="ExternalOutput")

    with tile.TileContext(nc) as tc:
        with (
            tc.tile_pool(name="stat", bufs=1) as stat,
            tc.tile_pool(name="dram", bufs=1, space="DRAM") as dpool,
        ):
            # scratch DRAM
            xh_dram = dpool.tile([T_ENC, FH, BC], DT, tag="xh_dram")
            lvraw = dpool.tile([T_DEC * 4, CW], F32, tag="lvraw")

            # ---- persistent SBUF tensors ----
            wink = stat.tile([8, FH], DT, tag="wink")
            winb = stat.tile([128, 1], F32, tag="winb")
            enck = stat.tile([65, 4 * H], DT, tag="enck")
            encrk = stat.tile([128, 2, 4 * H], DT, tag="encrk")
            deck = stat.tile([128, 2, 4 * H], DT, tag="deck")
            deckb = stat.tile([1, 4 * H], DT, tag="deckb")
            decrk = stat.tile([128, 2, 4 * H], DT, tag="decrk")
            whead = stat.tile([128, 2, 4], DT, tag="whead")
            hbias = stat.tile([1, 4], DT, tag="hbias")
            ident = stat.tile([128, 128], DT, tag="ident")
            ones = stat.tile([1, BC], DT, tag="ones")
            hT = stat.tile([128, 2, BC], DT, tag="hT")
            cT = stat.tile([128, 2, BC], F32, tag="cT")
            zdx = stat.tile([128, 8, BC], DT, tag="zdx")
            xh_buf = [
                stat.tile([65, BC], DT, tag=f"xh{i}", name=f"xh{i}") for i in range(2)
            ]

            nc.sync.dma_start(out=wink, in_=wink_d[:, :])
            nc.sync.dma_start(out=winb, in_=winb_d[:, :])
            nc.sync.dma_start(out=enck, in_=enck_d[:, :])
            nc.sync.dma_start(out=encrk, in_=encrk_d.ap().rearrange("k p m -> p k m"))
            nc.sync.dma_start(out=deck, in_=deck_d.ap().rearrange("k p m -> p k m"))
            nc.sync.dma_start(out=deckb, in_=deckb_d[:, :])
            nc.sync.dma_start(out=decrk, in_=decrk_d.ap().rearrange("k p m -> p k m"))
            nc.sync.dma_start(out=whead, in_=whead_d.ap().rearrange("k p m -> p k m"))
            nc.sync.dma_start(out=hbias, in_=hbias_d[:, :])
            nc.sync.dma_start(out=ident, in_=ident_d[:, :])
            nc.sync.dma_start(out=ones, in_=ones_d[:, :])
            for i in range(2):
                nc.sync.dma_start(out=xh_buf[i][64:65, :], in_=ones_d[:, :])
            nc.sync.dma_start(out=hT, in_=zeros_d.ap())
            nc.vector.memset(cT, 0.0)

            # ---- P1: input projection xh = relu(x @ w_in_k + b), transposed ----
            # col-packed: two 512-col chunks share the PE array (cols 0-63 / 64-127)
            with (
                tc.tile_pool(name="p1sb", bufs=4) as p1sb,
                tc.tile_pool(name="p1ps", bufs=2, space="PSUM") as p1ps,
                tc.tile_pool(name="p1ev", bufs=2) as p1ev,
            ):
                nchunks = T_ENC * NCH  # 40 (t, half) chunks
                for g in range(nchunks // 4):  # 10 groups of 4 chunks
                    pin = p1ps.tile([64, 4, CW], F32, tag="pin")
                    for bk in range(4):
                        j = g * 4 + bk
                        t, half = j // NCH, j % NCH
                        xc = p1sb.tile([8, CW], DT, tag="xc")
                        nc.sync.dma_start(out=xc, in_=xt_d[t, :, ds(half * CW, CW)])
                        nc.tensor.matmul(
                            pin[:, bk, :], wink[:, :], xc[:, :], start=True, stop=True
                        )
                    xh_sb = p1ev.tile([64, 4, CW], DT, tag="xh_sb")
                    nc.scalar.activation(
                        out=xh_sb, in_=pin, func=AF.Relu, bias=winb[0:64, :], scale=1.0
                    )
                    for bk in range(4):
                        j = g * 4 + bk
                        t, half = j // NCH, j % NCH
                        nc.sync.dma_start(
                            out=xh_dram[t, :, ds(half * CW, CW)],
                            in_=xh_sb[:, bk, :],
                        )

            # ---- scan-phase pools ----
            with (
                tc.tile_pool(name="psA", bufs=1, space="PSUM") as psA,
                tc.tile_pool(name="psB", bufs=1, space="PSUM") as psB,
                tc.tile_pool(name="psC", bufs=1, space="PSUM") as psC,
                tc.tile_pool(name="gsb", bufs=2) as gsb,
            ):

                def lstm_step(xparts, rk, t_idx, head=False):
                    """One LSTM step. xparts(m, cs) emits the start=True matmul
                    for m-tile m / chunk slice cs into the given psum AP."""
                    for c in range(NCH):
                        cs = ds(c * CW, CW)
                        pif = psA.tile([128, 4, CW], F32, tag="pif")
                        pg = psB.tile([128, 2, CW], F32, tag="pg")
                        po = psC.tile([128, 2, CW], F32, tag="po")
                        banks = [pif[:, j, :] for j in range(4)] + [
                            pg[:, j, :] for j in range(2)
                        ] + [po[:, j, :] for j in range(2)]
                        for m in range(8):
                            pt = banks[m]
                            xparts(pt, m, cs)
                            nc.tensor.matmul(
                                pt, rk[:, 0, ts(m, 128)], hT[:, 0, cs],
                                start=False, stop=False,
                            )
                            nc.tensor.matmul(
                                pt, rk[:, 1, ts(m, 128)], hT[:, 1, cs],
                                start=False, stop=True,
                            )
                        g_if = gsb.tile([128, 4, CW], F32, tag="g_if")
                        g_g = gsb.tile([128, 2, CW], F32, tag="g_g")
                        g_o = gsb.tile([128, 2, CW], F32, tag="g_o")
                        nc.scalar.activation(out=g_if, in_=pif, func=AF.Sigmoid)
                        nc.scalar.activation(out=g_g, in_=pg, func=AF.Tanh)
                        nc.scalar.activation(out=g_o, in_=po, func=AF.Sigmoid)
                        ig = gsb.tile([128, 2, CW], F32, tag="ig")
                        tc_t = gsb.tile([128, 2, CW], F32, tag="tc_t")
                        cc = cT[:, :, cs]
                        nc.vector.tensor_mul(ig, g_if[:, 0:2, :], g_g)
                        nc.vector.tensor_mul(cc, g_if[:, 2:4, :], cc)
                        nc.vector.tensor_add(cc, cc, ig)
                        nc.scalar.activation(out=tc_t, in_=cc, func=AF.Tanh)
                        nc.vector.tensor_mul(hT[:, :, cs], g_o, tc_t)
                        if head:
                            # one PSUM bank, reusing po's slot after its drain
                            ph = psC.tile([4, CW], F32, tag="po")
                            nc.tensor.matmul(
                                ph[:, :], whead[:, 0, :], hT[:, 0, cs],
                                start=True, stop=False,
                            )
                            nc.tensor.matmul(
                                ph[:, :], whead[:, 1, :], hT[:, 1, cs],
                                start=False, stop=False,
                            )
                            nc.tensor.matmul(
                                ph[:, :], hbias[:, :], ones[:, 0:CW],
                                start=False, stop=True,
                            )
                            # row layout of om/lvraw: t*4 + unit*2 + chunk
                            om_v = om_d.ap().rearrange(
                                "(a p h) n -> a p h n", p=2, h=2
                            )
                            lv_v = lvraw[:, :].rearrange(
                                "(a p h) n -> a p h n", p=2, h=2
                            )
                            ph_sb = gsb.tile([4, CW], F32, tag="ph_sb")
                            nc.vector.tensor_copy(ph_sb, ph)
                            nc.sync.dma_start(
                                out=om_v[t_idx, :, c, :], in_=ph_sb[0:2, :]
                            )
                            nc.sync.dma_start(
                                out=lv_v[t_idx, :, c, :], in_=ph_sb[2:4, :]
                            )

                # ---- P2: encoder ----
                for t in range(T_ENC):
                    xh = xh_buf[t % 2]
                    nc.sync.dma_start(out=xh[0:64, :], in_=xh_dram[t, :, :])

                    def xp(pt, m, cs, xh=xh):
                        nc.tensor.matmul(
                            pt, enck[:, ts(m, 128)], xh[:, cs], start=True, stop=False
                        )

                    lstm_step(xp, encrk, t)

                # ---- P3: zdx = dec_k.T @ enc + dec_b (once) ----
                for c in range(NCH):
                    cs = ds(c * CW, CW)
                    for g in range(2):
                        pz = psA.tile([128, 4, CW], F32, tag="pif")
                        for mi in range(4):
                            m = g * 4 + mi
                            nc.tensor.matmul(
                                pz[:, mi, :], deck[:, 0, ts(m, 128)], hT[:, 0, cs],
                                start=True, stop=False,
                            )
                            nc.tensor.matmul(
                                pz[:, mi, :], deck[:, 1, ts(m, 128)], hT[:, 1, cs],
                                start=False, stop=False,
                            )
                            nc.tensor.matmul(
                                pz[:, mi, :], deckb[:, ts(m, 128)], ones[:, 0:CW],
                                start=False, stop=True,
                            )
                        nc.scalar.activation(
                            out=zdx[:, ds(g * 4, 4), cs], in_=pz, func=AF.Copy
                        )
                nc.sync.dma_start(out=hT, in_=zeros_d.ap())
                nc.vector.memset(cT, 0.0)

                # ---- P4: decoder ----
                for t in range(T_DEC):

                    def xp(pt, m, cs):
                        nc.tensor.matmul(
                            pt, ident[:, :], zdx[:, m, cs], start=True, stop=False
                        )

                    lstm_step(xp, decrk, t, head=True)

                # ---- P5: relu(log_var) ----
                with tc.tile_pool(name="p5", bufs=1) as p5:
                    lv_sb = p5.tile([T_DEC * 4, CW], F32, tag="lv_sb")
                    nc.sync.dma_start(out=lv_sb, in_=lvraw[:, :])
                    nc.scalar.activation(out=lv_sb, in_=lv_sb, func=AF.Relu)
                    nc.sync.dma_start(out=ol_d[:, :], in_=lv_sb)

    nc.compile()
    return nc


def _marshal(x, w_in_k, w_in_b, enc_k, enc_rk, enc_b,
             dec_k, dec_rk, dec_b, mean_k, mean_b, lv_k, lv_b):
    f = np.float32
    x = np.asarray(x, f)
    enck_ext = np.concatenate([np.asarray(enc_k, f), np.asarray(enc_b, f)[None, :]], 0)
    shared = {
        "w_in_k": np.ascontiguousarray(np.asarray(w_in_k, f)),
        "w_in_b128": np.ascontiguousarray(
            np.tile(np.asarray(w_in_b, f), 2)[:, None]
        ),
        "enc_k_ext": np.ascontiguousarray(enck_ext),
        "enc_rk": np.ascontiguousarray(np.asarray(enc_rk, f).reshape(2, 128, 4 * H)),
        "dec_k": np.ascontiguousarray(np.asarray(dec_k, f).reshape(2, 128, 4 * H)),
        "dec_b": np.ascontiguousarray(np.asarray(dec_b, f)[None, :]),
        "dec_rk": np.ascontiguousarray(np.asarray(dec_rk, f).reshape(2, 128, 4 * H)),
        "w_head": np.ascontiguousarray(
            np.concatenate([np.asarray(mean_k, f), np.asarray(lv_k, f)], 1).reshape(
                2, 128, 4
            )
        ),
        "head_bias": np.ascontiguousarray(
            np.concatenate([np.asarray(mean_b, f), np.asarray(lv_b, f)])[None, :]
        ),
        "ident": np.eye(128, dtype=f),
        "ones": np.ones((1, BC), f),
        "zeros": np.zeros((128, 2, BC), f),
    }
    in_maps = []
    for c in range(N_CORES):
        xs = x[c * BC : (c + 1) * BC]  # (BC, 20, 8)
        m = dict(shared)
        m["xt"] = np.ascontiguousarray(xs.transpose(1, 2, 0))  # (20, 8, BC)
        in_maps.append(m)
    return in_maps


def _assemble(results):
    outs = []
    for c in range(N_CORES):
        om = results[c]["out_mean"].reshape(T_DEC, 2, BC)  # (t, o, b)
        ol = results[c]["out_lv"].reshape(T_DEC, 2, BC)
        o = np.concatenate(
            [om.transpose(2, 0, 1), ol.transpose(2, 0, 1)], axis=2
        )  # (BC, 15, 4)
        outs.append(o)
    return np.ascontiguousarray(np.concatenate(outs, 0))


def _run(trace=False, **inputs):
    global LAST_RESULTS
    if not _NC_CACHE:
        _NC_CACHE.append(_build_nc())
    nc = _NC_CACHE[0]
    in_maps = _marshal(**inputs)
    LAST_RESULTS = bass_utils.run_bass_kernel_spmd(
        nc, in_maps, core_ids=list(range(N_CORES)), trace=trace
    )
    return _assemble(LAST_RESULTS.results)


def kernel(**inputs):
    return _run(trace=False, **inputs)

